# revision 1
# baseline (speedup 1.0000x reference)
"""Trainium2 Bass kernel for nn_AttentionBlock (S=4096, H=1024, NH=2, DS=64).

Strategy (v2): full sequence parallelism. Each core owns 512 rows (queries
AND keys): it computes Q^T, K^T, V for its own block only (bf16 operands),
then the K^T/V shards are exchanged with 4 chunked AllGathers (one per
key-quarter) so attention overlaps the collectives. Full K^T and V stay
SBUF-resident in bf16 (64KB + 64KB per partition). Heads are interleaved
per key-quarter; ctx partial sums accumulate in SBUF f32 (PSUM holds the
two lsum rows for the whole sweep). Out-projection + residual + LayerNorm
as in v1.
"""

import math
import sys

sys.path.insert(0, "/opt/trn_rl_repo")

import numpy as np
import ml_dtypes

import concourse.bass as bass
import concourse.mybir as mybir
import concourse.tile as tile
from concourse import bacc
from concourse.bass_utils import run_bass_kernel_spmd

S, H, NH, DS = 4096, 1024, 2, 64
HD = H // NH            # 512
NC = 8                  # cores
SQ = S // NC            # 512 queries (and keys) per core
EPS = 1e-5
F32 = mybir.dt.float32
F32R = mybir.dt.float32r
BF16 = mybir.dt.bfloat16
AF = mybir.ActivationFunctionType
ALU = mybir.AluOpType

KC = S // 128           # 32 key chunks of 128 (global)
HC = H // 128           # 8 hidden chunks of 128
QB = SQ // 128          # 4 query chunks of 128
NJ = 4                  # key sub-chunks per core block (AG quarters)
AGW = HC * 128 + H      # 2048 bf16 elems per partition per AG quarter


def build_program():
    nc = bacc.Bacc("TRN2", target_bir_lowering=False, debug=False, num_devices=NC)

    # ---- DRAM I/O ----
    xq = nc.dram_tensor("xq", [SQ, H], F32, kind="ExternalInput")
    wqT = nc.dram_tensor("wqT", [H, H], BF16, kind="ExternalInput")
    wkT = nc.dram_tensor("wkT", [H, H], BF16, kind="ExternalInput")
    wvT = nc.dram_tensor("wvT", [H, H], BF16, kind="ExternalInput")
    woT = nc.dram_tensor("woT", [H, H], BF16, kind="ExternalInput")
    wsT = nc.dram_tensor("wsT", [DS, H], F32R, kind="ExternalInput")
    sdat = nc.dram_tensor("sdat", [DS, 1], F32R, kind="ExternalInput")
    bsv = nc.dram_tensor("bsv", [H], F32, kind="ExternalInput")
    mbias = nc.dram_tensor("mbias", [128, KC], F32, kind="ExternalInput")
    onescol = nc.dram_tensor("onescol", [128, 1], BF16, kind="ExternalInput")
    onesrow = nc.dram_tensor("onesrow", [1, 128], BF16, kind="ExternalInput")
    identd = nc.dram_tensor("identd", [128, 128], F32R, kind="ExternalInput")
    lnw = nc.dram_tensor("lnw", [H], F32, kind="ExternalInput")
    lnb = nc.dram_tensor("lnb", [H], F32, kind="ExternalInput")
    out = nc.dram_tensor("out", [SQ, H], F32, kind="ExternalOutput")

    inv_sqrt_hd = 1.0 / math.sqrt(HD)

    with tile.TileContext(nc) as tc:
        with (
            tc.tile_pool(name="consts", bufs=1) as consts,
            tc.tile_pool(name="persist", bufs=1) as persist,
            tc.tile_pool(name="rlp", bufs=1) as rlp,
            tc.tile_pool(name="dram", bufs=1, space="DRAM") as dram,
        ):
            # ---- constants; sync queue carries only the stage-1-critical
            # ones (semb chain + transpose ident + x rows), the rest ride
            # the gpsimd queue which is otherwise idle until the collectives.
            sd_sb = consts.tile([DS, 1], F32R)
            nc.sync.dma_start(sd_sb, sdat[:, :])
            wsT_sb = consts.tile([DS, H], F32R)
            nc.sync.dma_start(wsT_sb, wsT[:, :])
            ident = consts.tile([128, 128], F32R)
            nc.sync.dma_start(ident, identd[:, :])
            Af = consts.tile([128, 36], F32)     # 0:32 maskbias | 32 zero | 33 eps
            mb_sb = Af[:, 0:32]
            nc.gpsimd.dma_start(mb_sb, mbias[:, :])
            zb_sb = Af[:, 32:33]
            nc.vector.memset(zb_sb, 0.0)
            eps_sb = Af[:, 33:34]
            nc.vector.memset(eps_sb, EPS)
            ones_sb = consts.tile([128, 1], BF16)
            nc.gpsimd.dma_start(ones_sb, onescol[:, :])
            onesrow_sb = consts.tile([1, 128], BF16)
            nc.gpsimd.dma_start(onesrow_sb, onesrow[:, :])
            lnw_b = consts.tile([128, H], F32)
            lnb_b = consts.tile([128, H], F32)

            # ---- persistent tiles ----
            qT_sb = persist.tile([128, HC, SQ], BF16)      # Q^T/sqrt(hd): [d, q]
            ctxT = persist.tile([128, HC, SQ], BF16)       # ctx^T/l: [d, q]
            wo_sb = persist.tile([128, HC, H], BF16)
            vb_bcast = persist.tile([128, H], F32)
            semb_bf = persist.tile([128, HC], BF16)

            # DRAM scratch
            semb_scr = dram.tile([H], F32)
            vb_scr = dram.tile([H], F32)
            kvin = dram.tile([NJ, 128, AGW], BF16)
            agouts = [dram.tile([NC, 128, AGW], BF16, addr_space="Shared",
                                name=f"agout{j}")
                      for j in range(NJ)]

            # warm-up collective: absorbs the cross-core rendezvous/launch
            # skew while stage 1 computes, so the first real AllGather runs
            # at steady state
            warm_in = dram.tile([1, 32], F32)
            warm_out = dram.tile([NC, 1, 32], F32, addr_space="Shared")
            nc.gpsimd.dma_start(warm_in[:], mbias[0:1, 0:32])
            nc.gpsimd.collective_compute(
                "AllGather", mybir.AluOpType.bypass,
                replica_groups=[list(range(NC))],
                ins=[warm_in.opt()], outs=[warm_out.opt()],
            )

            # ================= Stage 1: own-block projections ==================
            # Order targets the critical path to the first AllGather: K^T and
            # V shards first (ship each key-quarter as soon as ready), Q^T
            # last (it overlaps AllGather 0).
            with (
                tc.tile_pool(name="s1", bufs=1) as s1,
                tc.tile_pool(name="xtp", bufs=2) as xtp,
                tc.tile_pool(name="ps1", bufs=3, space="PSUM") as ps1,
                tc.tile_pool(name="pst", bufs=2, space="PSUM") as pst,
                tc.tile_pool(name="psb", bufs=2, space="PSUM") as psb,
            ):
                wk_sb = s1.tile([128, HC, H], BF16, tag="wk")
                nc.scalar.dma_start(wk_sb, wkT.rearrange("(c p) d -> p c d", p=128))
                wv_sb = s1.tile([128, HC, H], BF16, tag="wv")
                nc.scalar.dma_start(wv_sb, wvT.rearrange("(c p) d -> p c d", p=128))
                wq_sb = s1.tile([128, HC, H], BF16, tag="wq")
                nc.scalar.dma_start(wq_sb, wqT.rearrange("(c p) d -> p c d", p=128))
                nc.scalar.dma_start(wo_sb, woT.rearrange("(c p) d -> p c d", p=128))
                xT_sb = s1.tile([128, HC, SQ], BF16, tag="xT")
                ktown = s1.tile([128, HC, SQ], BF16, tag="ktown")
                vown = s1.tile([128, NJ, H], BF16, tag="vown")

                # --- transpose own x rows -> x^T (bf16) ---
                for qb in range(QB):
                    xin = xtp.tile([128, H], F32R, tag="xin", bufs=3, name=f"xin{qb}")
                    nc.sync.dma_start(
                        xin[:, 0:512],
                        xq[qb * 128:(qb + 1) * 128, 0:512].bitcast(F32R))
                    nc.gpsimd.dma_start(
                        xin[:, 512:H],
                        xq[qb * 128:(qb + 1) * 128, 512:H].bitcast(F32R))
                    for hc in range(HC):
                        pt = pst.tile([128, 128], F32R, tag="ptr", name=f"tr{qb}_{hc}")
                        nc.tensor.transpose(
                            pt[:], xin[:, hc * 128:(hc + 1) * 128], ident)
                        nc.any.tensor_copy(xT_sb[:, hc, qb * 128:(qb + 1) * 128],
                                           pt[:].bitcast(F32))

                # --- semb = Ws @ static + bs; then bias rows (roundtrips) ---
                bs_row = rlp.tile([1, H], F32, tag="row", name="bs_row")
                nc.scalar.dma_start(bs_row, bsv.rearrange("d -> () d"))
                semb_row = rlp.tile([1, H], F32, tag="srow")
                for d2 in range(H // 512):
                    p = psb.tile([1, 512], F32, tag="pbias", name=f"sembp{d2}")
                    nc.tensor.matmul(p[:], sd_sb[:], wsT_sb[:, d2 * 512:(d2 + 1) * 512],
                                     start=True, stop=True)
                    nc.vector.tensor_add(semb_row[:, d2 * 512:(d2 + 1) * 512], p[:],
                                         bs_row[:, d2 * 512:(d2 + 1) * 512])
                nc.scalar.dma_start(semb_scr.rearrange("d -> () d"), semb_row[:])
                semb_pc = rlp.tile([128, HC], F32, tag="spc")
                nc.scalar.dma_start(semb_pc, semb_scr.rearrange("(c p) -> p c", p=128))
                nc.vector.tensor_copy(semb_bf, semb_pc)

                # --- K^T own first (needs only wk + x^T); kbias is skipped:
                # Wk@semb adds a per-query constant to the logits, which
                # cancels in softmax ---
                for dc in range(HC):
                    p = ps1.tile([128, SQ], F32, tag="pproj", name=f"kp{dc}")
                    for hc in range(HC):
                        nc.tensor.matmul(p[:], wk_sb[:, hc, dc * 128:(dc + 1) * 128],
                                         xT_sb[:, hc, :],
                                         start=(hc == 0), stop=(hc == HC - 1))
                    nc.scalar.copy(ktown[:, dc, :], p[:])

                # --- vbias row ---
                vb_row = rlp.tile([1, H], F32, tag="row", name="vb_row")
                for d2 in range(H // 512):
                    p = psb.tile([1, 512], F32, tag="pbias", name=f"vbp{d2}")
                    for hc in range(HC):
                        nc.tensor.matmul(p[:], semb_bf[:, hc:hc + 1],
                                         wv_sb[:, hc, d2 * 512:(d2 + 1) * 512],
                                         start=(hc == 0), stop=(hc == HC - 1))
                    nc.vector.tensor_copy(vb_row[:, d2 * 512:(d2 + 1) * 512], p[:])
                nc.scalar.dma_start(vb_scr.rearrange("d -> () d"), vb_row[:])
                nc.scalar.dma_start(vb_bcast,
                                    bass.AP(tensor=vb_scr.tensor, offset=vb_scr.offset,
                                            ap=[[0, 128], [1, H]]))
                for j in range(NJ):
                    for d2 in range(H // 512):
                        p = ps1.tile([128, 512], F32, tag="pproj", name=f"vp{j}_{d2}")
                        for hc in range(HC):
                            nc.tensor.matmul(p[:], xT_sb[:, hc, j * 128:(j + 1) * 128],
                                             wv_sb[:, hc, d2 * 512:(d2 + 1) * 512],
                                             start=(hc == 0), stop=(hc == HC - 1))
                        nc.vector.tensor_add(vown[:, j, d2 * 512:(d2 + 1) * 512], p[:],
                                             vb_bcast[:, d2 * 512:(d2 + 1) * 512])
                    nc.gpsimd.dma_start(
                        kvin[j, :, 0:HC * 128].rearrange("p (dc k) -> p dc k", dc=HC),
                        ktown[:, :, j * 128:(j + 1) * 128])
                    nc.gpsimd.dma_start(kvin[j, :, HC * 128:AGW], vown[:, j, :])
                for dc in range(HC):
                    p = ps1.tile([128, SQ], F32, tag="pproj", name=f"qp{dc}")
                    for hc in range(HC):
                        nc.tensor.matmul(p[:], wq_sb[:, hc, dc * 128:(dc + 1) * 128],
                                         xT_sb[:, hc, :],
                                         start=(hc == 0), stop=(hc == HC - 1))
                    nc.scalar.mul(qT_sb[:, dc, :], p[:], inv_sqrt_hd)

            # ============ Stage 2: chunked AllGather + attention ==============
            with (
                tc.tile_pool(name="kv", bufs=1) as kv,
                tc.tile_pool(name="attn", bufs=1) as attn,
                tc.tile_pool(name="ps_s", bufs=2, space="PSUM") as ps_s,
                tc.tile_pool(name="ps_c", bufs=1, space="PSUM") as ps_c,
                tc.tile_pool(name="ps_l", bufs=1, space="PSUM") as ps_l,
            ):
                # [p(d), j, c, dc*128+k]: matches the AllGather output layout
                kT_full = kv.tile([128, NJ, NC, HC * 128], BF16, tag="kT")
                v_full = kv.tile([128, KC, H], BF16, tag="v")      # [k, d]
                ctx_acc = kv.tile([128, HC, SQ], F32, tag="ctxa")  # unnormalized

                for j in range(NJ):
                    nc.gpsimd.collective_compute(
                        "AllGather",
                        mybir.AluOpType.bypass,
                        replica_groups=[list(range(NC))],
                        ins=[kvin[j].opt()],
                        outs=[agouts[j].opt()],
                    )
                for j in range(NJ):
                    for c in range(NC):
                        nc.sync.dma_start(
                            kT_full[:, j, c, :],
                            agouts[j][c, :, 0:HC * 128])
                        nc.gpsimd.dma_start(
                            v_full[:, c * NJ + j, :],
                            agouts[j][c, :, HC * 128:AGW])

                lsums = [ps_l.tile([1, SQ], F32, tag=f"l{h}", name=f"lsum{h}")
                         for h in range(NH)]

                def normalize(h):
                    # ctxT_h = ctx_acc_h / l_h (reciprocal broadcast via PE)
                    lrow = rlp.tile([1, SQ], BF16, tag="rl", name=f"lrow{h}")
                    nc.scalar.copy(lrow[:], lsums[h][:])
                    lb_ps = ps_s.tile([128, SQ], F32, tag="st", name=f"lbps{h}")
                    nc.tensor.matmul(lb_ps[:], onesrow_sb[:], lrow[:],
                                     start=True, stop=True)
                    rl_b = rlp.tile([128, SQ], F32, tag="rlb", name=f"rlb{h}")
                    nc.vector.reciprocal(rl_b[:], lb_ps[:])
                    for dv in range(4):
                        nc.vector.tensor_mul(ctxT[:, 4 * h + dv, :],
                                             ctx_acc[:, 4 * h + dv, :], rl_b[:])

                for j in range(NJ):
                    for h in range(NH):
                        ctx_ps = [ps_c.tile([128, SQ], F32, tag=f"ctx{dv}",
                                            name=f"ctxps{j}_{h}_{dv}")
                                  for dv in range(4)]
                        PTs = {}

                        def consume(c, j=j, h=h, PTs=PTs, ctx_ps=ctx_ps):
                            PTk = PTs.pop(c)
                            nc.tensor.matmul(lsums[h][:], ones_sb, PTk[:],
                                             start=(j == 0 and c == 0),
                                             stop=(j == NJ - 1 and c == NC - 1),
                                             skip_group_check=True)
                            kc = c * NJ + j
                            for dv in range(4):
                                nc.tensor.matmul(ctx_ps[dv][:],
                                                 v_full[:, kc,
                                                        (4 * h + dv) * 128:
                                                        (4 * h + dv + 1) * 128],
                                                 PTk[:],
                                                 start=(c == 0), stop=(c == NC - 1),
                                                 skip_group_check=True)

                        for c in range(NC):
                            kc = c * NJ + j     # global 128-key chunk id
                            ps = ps_s.tile([128, SQ], F32, tag="st",
                                           name=f"st{j}_{h}_{c}")
                            for dq in range(4):
                                nc.tensor.matmul(
                                    ps[:],
                                    kT_full[:, j, c,
                                            (4 * h + dq) * 128:(4 * h + dq + 1) * 128],
                                    qT_sb[:, 4 * h + dq, :],
                                    start=(dq == 0), stop=(dq == 3))
                            PTk = attn.tile([128, SQ], BF16, tag="PTs", bufs=3,
                                            name=f"PT{j}_{h}_{c}")
                            PTs[c] = PTk
                            bias_ap = mb_sb[:, kc:kc + 1] if h == 0 else zb_sb
                            nc.scalar.activation(PTk[:], ps[:], AF.Exp, bias=bias_ap)
                            if c > 0:
                                consume(c - 1)
                        consume(NC - 1)

                        # fold this quarter's ctx into the f32 accumulator
                        for dv in range(4):
                            dst = ctx_acc[:, 4 * h + dv, :]
                            if j == 0:
                                nc.vector.tensor_copy(dst, ctx_ps[dv][:])
                            else:
                                nc.vector.tensor_add(dst, dst, ctx_ps[dv][:])
                        if j == NJ - 1:
                            normalize(h)

            # ---- Stage 3: out-proj (natural layout) + residual + LN ----
            with (
                tc.tile_pool(name="s4", bufs=2) as s4,
                tc.tile_pool(name="ps4", bufs=2, space="PSUM") as ps4,
            ):
                nc.gpsimd.dma_start(
                    lnw_b, bass.AP(tensor=lnw, offset=0, ap=[[0, 128], [1, H]]))
                nc.gpsimd.dma_start(
                    lnb_b, bass.AP(tensor=lnb, offset=0, ap=[[0, 128], [1, H]]))
                for qb in range(QB):
                    xq_f = s4.tile([128, H], F32, tag="xqf", name=f"xqf{qb}")
                    nc.sync.dma_start(xq_f, xq[qb * 128:(qb + 1) * 128, :])
                    res_f = s4.tile([128, H], F32, tag="resf", name=f"resf{qb}")
                    for h2 in range(H // 512):
                        p = ps4.tile([128, 512], F32, tag="pout", name=f"po{qb}_{h2}")
                        for dc in range(HC):
                            nc.tensor.matmul(p[:],
                                             ctxT[:, dc, qb * 128:(qb + 1) * 128],
                                             wo_sb[:, dc, h2 * 512:(h2 + 1) * 512],
                                             start=(dc == 0), stop=(dc == HC - 1))
                        nc.vector.tensor_add(res_f[:, h2 * 512:(h2 + 1) * 512], p[:],
                                             xq_f[:, h2 * 512:(h2 + 1) * 512])
                    # LayerNorm via bn_stats
                    LS = s4.tile([128, 16], F32, tag="lns", name=f"lns{qb}")
                    for h2 in range(H // 512):
                        nc.vector.bn_stats(
                            LS[:, h2 * 6:(h2 + 1) * 6]
                            .rearrange("p (a b) -> p a b", a=1),
                            res_f[:, h2 * 512:(h2 + 1) * 512])
                    nc.vector.bn_aggr(LS[:, 12:14], LS[:, 0:12]
                                      .rearrange("p (a b) -> p a b", a=2))
                    nc.scalar.activation(LS[:, 14:15], LS[:, 13:14], AF.Sqrt,
                                         bias=eps_sb)
                    nc.vector.reciprocal(LS[:, 15:16], LS[:, 14:15])
                    norm = s4.tile([128, H], F32, tag="norm", name=f"norm{qb}", bufs=1)
                    scl = s4.tile([128, H], F32, tag="scl", name=f"scl{qb}", bufs=1)
                    fin = s4.tile([128, H], F32, tag="fin", name=f"fin{qb}")
                    for h2 in range(H // 512):
                        sl = slice(h2 * 512, (h2 + 1) * 512)
                        nc.vector.tensor_scalar(norm[:, sl], res_f[:, sl],
                                                LS[:, 12:13], LS[:, 15:16],
                                                ALU.subtract, ALU.mult)
                        nc.vector.tensor_mul(scl[:, sl], norm[:, sl], lnw_b[:, sl])
                        nc.vector.tensor_add(fin[:, sl], scl[:, sl], lnb_b[:, sl])
                        nc.sync.dma_start(out[qb * 128:(qb + 1) * 128, sl],
                                          fin[:, sl])

    nc.compile()
    return nc


_CACHED_NC = None


def _get_nc():
    global _CACHED_NC
    if _CACHED_NC is None:
        _CACHED_NC = build_program()
    return _CACHED_NC


def _prep_inputs(inputs, static_data, base_mask, Wq, Wk, Wv, Wo, Ws, bs, ln_w, ln_b):
    f32 = np.float32
    bf16 = ml_dtypes.bfloat16
    xf = np.ascontiguousarray(inputs, f32)
    common = {
        "wqT": np.ascontiguousarray(np.asarray(Wq, f32).T).astype(bf16),
        "wkT": np.ascontiguousarray(np.asarray(Wk, f32).T).astype(bf16),
        "wvT": np.ascontiguousarray(np.asarray(Wv, f32).T).astype(bf16),
        "woT": np.ascontiguousarray(np.asarray(Wo, f32).T).astype(bf16),
        "wsT": np.ascontiguousarray(np.asarray(Ws, f32).T),
        "sdat": np.ascontiguousarray(np.asarray(static_data, f32).reshape(DS, 1)),
        "bsv": np.ascontiguousarray(bs, f32),
        "mbias": np.ascontiguousarray(
            np.where(np.asarray(base_mask, bool), 0.0, -1e30)
            .astype(f32).reshape(KC, 128).T),
        "onescol": np.ones((128, 1), bf16),
        "onesrow": np.ones((1, 128), bf16),
        "identd": np.eye(128, dtype=f32),
        "lnw": np.ascontiguousarray(ln_w, f32),
        "lnb": np.ascontiguousarray(ln_b, f32),
    }
    in_maps = []
    for c in range(NC):
        m = dict(common)
        m["xq"] = np.ascontiguousarray(xf[c * SQ:(c + 1) * SQ, :])
        in_maps.append(m)
    return in_maps


def kernel_run(trace=False, **inputs):
    nc = _get_nc()
    in_maps = _prep_inputs(**inputs)
    res = run_bass_kernel_spmd(nc, in_maps, core_ids=list(range(NC)), trace=trace)
    outp = np.concatenate([res.results[c]["out"] for c in range(NC)], axis=0)
    return outp, res


def kernel(**inputs):
    outp, _ = kernel_run(trace=False, **inputs)
    return outp



# revision 15
# speedup vs baseline: 1.5198x; 1.5198x over previous
"""Trainium2 Bass kernel for nn_AttentionBlock (S=4096, H=1024, NH=2, DS=64).

Strategy (v4): sequence parallelism; bf16 projections with fp8 (e4m3)
DoubleRow attention. Each core owns 512 rows (queries AND keys). Host ships
x^T bf16 and weights bf16 in the exact SBUF layouts plus vb = Wv@semb (the
K-side semb constant cancels in softmax and is dropped; the V-side enters
as a ones-row matmul into the V psum group). Projection PSUM results are
quantized to fp8 on copy-out; K^T and V shards are exchanged with 4 chunked
fp8 AllGathers (K head0, K head1, V dhalf0, V dhalf1). QK, PV and the lsum
run as fp8 DoubleRow matmuls (contract 256/instr); exp(logits - 3.5) on the
scalar engine keeps everything inside e4m3 range (exact softmax invariance).
ctx accumulates in PSUM, is normalized to bf16, and the out-projection +
residual + LayerNorm run in bf16/f32 as in the reference.
"""

import math
import sys

sys.path.insert(0, "/opt/trn_rl_repo")

import numpy as np
import ml_dtypes

import concourse.bass as bass
import concourse.mybir as mybir
import concourse.tile as tile
from concourse import bacc
from concourse.bass_utils import run_bass_kernel_spmd

S, H, NH, DS = 4096, 1024, 2, 64
HD = H // NH            # 512
NC = 8                  # cores
SQ = S // NC            # 512 queries (and keys) per core
EPS = 1e-5
F32 = mybir.dt.float32
BF16 = mybir.dt.bfloat16
F8 = mybir.dt.float8e4
AF = mybir.ActivationFunctionType
ALU = mybir.AluOpType
PM = mybir.MatmulPerfMode.DoubleRow

KC = S // 128           # 32 global key chunks of 128
HC = H // 128           # 8 hidden chunks of 128
QB = SQ // 128          # 4 query chunks of 128
SC = 1.0 / math.sqrt(HD)               # exp input scale
MSHIFT = 3.5            # uniform logit shift: keeps exp() in fp8 range (exact)


def build_program(affine: bool):
    nc = bacc.Bacc("TRN2", target_bir_lowering=False, debug=False, num_devices=NC)

    # ---- DRAM I/O (host pre-arranged layouts) ----
    xTd = nc.dram_tensor("xTd", [128, HC, SQ], BF16, kind="ExternalInput")
    wqd = nc.dram_tensor("wqd", [128, HC, H], BF16, kind="ExternalInput")
    wkd = nc.dram_tensor("wkd", [128, HC, H], BF16, kind="ExternalInput")
    wvd = nc.dram_tensor("wvd", [128, HC, H], BF16, kind="ExternalInput")
    wod = nc.dram_tensor("wod", [128, HC, H], BF16, kind="ExternalInput")
    vbd = nc.dram_tensor("vbd", [1, H], BF16, kind="ExternalInput")
    xqd = nc.dram_tensor("xqd", [SQ, H], F32, kind="ExternalInput")
    mbias = nc.dram_tensor("mbias", [128, KC], F32, kind="ExternalInput")
    onesrowd = nc.dram_tensor("onesrowd", [1, 128], BF16, kind="ExternalInput")
    if affine:
        lnw = nc.dram_tensor("lnw", [H], F32, kind="ExternalInput")
        lnb = nc.dram_tensor("lnb", [H], F32, kind="ExternalInput")
    out = nc.dram_tensor("out", [SQ, H], F32, kind="ExternalOutput")

    with tile.TileContext(nc) as tc:
        with (
            tc.tile_pool(name="consts", bufs=1) as consts,
            tc.tile_pool(name="persist", bufs=1) as persist,
            tc.tile_pool(name="rlp", bufs=1) as rlp,
            tc.tile_pool(name="dram", bufs=1, space="DRAM") as dram,
        ):
            # ---- constants ----
            Af = consts.tile([128, 36], F32)     # 0:32 maskbias | 32 -M | 33 eps
            mb_sb = Af[:, 0:32]
            nc.gpsimd.dma_start(mb_sb, mbias[:, :])
            zb_sb = Af[:, 32:33]
            nc.vector.memset(zb_sb, -MSHIFT)
            eps_sb = Af[:, 33:34]
            nc.vector.memset(eps_sb, EPS)
            onesrow_sb = consts.tile([1, 128], BF16)
            nc.gpsimd.dma_start(onesrow_sb, onesrowd[:, :])
            vb_sb = consts.tile([1, H], BF16)
            nc.gpsimd.dma_start(vb_sb, vbd[:, :])
            if affine:
                lnw_b = consts.tile([128, H], F32)
                lnb_b = consts.tile([128, H], F32)

            # ---- persistent tiles ----
            xT_sb = persist.tile([128, HC, SQ], BF16)
            wq_sb = persist.tile([128, HC, H], BF16)
            wo_sb = persist.tile([128, HC, H], BF16)
            qT_sb = persist.tile([128, HC, SQ], F8)        # Q^T fp8 [d, q]
            kT_full = persist.tile([128, HC, S], F8)       # K^T fp8 [d, k-global]
            # V fp8 [k, d] + trailing 128-col ones block (lsum weights)
            v_full = persist.tile([128, KC, H + 128], F8)
            ctxT = persist.tile([128, HC, SQ], BF16)       # ctx^T/l bf16 [d, q]

            # DRAM scratch for collectives
            kvK = [dram.tile([128, 4, SQ], F8, name=f"kvK{h}") for h in range(2)]
            agK = [dram.tile([NC, 128, 4, SQ], F8, addr_space="Shared",
                             name=f"agK{h}") for h in range(2)]
            kvV = [dram.tile([128, 4, HD], F8, name=f"kvV{h}") for h in range(2)]
            agV = [dram.tile([NC, 128, 4, HD], F8, addr_space="Shared",
                             name=f"agV{h}") for h in range(2)]

            # warm-up collective: absorbs cross-core rendezvous/launch skew
            warm_in = dram.tile([1, 32], F32)
            warm_out = dram.tile([NC, 1, 32], F32, addr_space="Shared")
            nc.gpsimd.dma_start(warm_in[:], mbias[0:1, 0:32])
            nc.gpsimd.collective_compute(
                "AllGather", mybir.AluOpType.bypass,
                replica_groups=[list(range(NC))],
                ins=[warm_in.opt()], outs=[warm_out.opt()],
            )
            nc.vector.memset(v_full[:, :, H:H + 128], 1.0)

            # ================= Stage 1: own-block projections =================
            # K projects plain x: the Wk@semb key-constant adds a per-query
            # offset to the logits which cancels in softmax. V gets Wv@semb
            # via a ones-row matmul with the host-computed vb row.
            with (
                tc.tile_pool(name="s1", bufs=1) as s1,
                tc.tile_pool(name="ps1", bufs=3, space="PSUM") as ps1,
            ):
                wk_sb = s1.tile([128, HC, H], BF16, tag="wk", name="wk_sb")
                wv_sb = s1.tile([128, HC, H], BF16, tag="wv", name="wv_sb")
                k_stage = [s1.tile([128, 4, SQ], F8, tag=f"ks{h}", name=f"ks{h}")
                           for h in range(2)]
                v_stage = [s1.tile([128, 4, HD], F8, tag=f"vs{h}", name=f"vs{h}")
                           for h in range(2)]

                # loads; wk/xT first (K^T gates the first AllGather)
                nc.sync.dma_start(xT_sb, xTd[:, :, :])
                nc.scalar.dma_start(wk_sb[:, :, 0:HD], wkd[:, :, 0:HD])
                nc.scalar.dma_start(wk_sb[:, :, HD:H], wkd[:, :, HD:H])
                nc.sync.dma_start(wq_sb[:, :, 0:HD], wqd[:, :, 0:HD])
                nc.gpsimd.dma_start(wv_sb[:, :, 0:HD], wvd[:, :, 0:HD])
                nc.sync.dma_start(wq_sb[:, :, HD:H], wqd[:, :, HD:H])
                nc.gpsimd.dma_start(wv_sb[:, :, HD:H], wvd[:, :, HD:H])

                # K^T per head-half of d; ship each half as soon as ready
                for h in range(2):
                    for dci in range(4):
                        dc = 4 * h + dci
                        p = ps1.tile([128, SQ], F32, tag="pp", name=f"kp{dc}")
                        for hc in range(HC):
                            nc.tensor.matmul(
                                p[:], wk_sb[:, hc, dc * 128:(dc + 1) * 128],
                                xT_sb[:, hc, :],
                                start=(hc == 0), stop=(hc == HC - 1))
                        nc.scalar.copy(k_stage[h][:, dci, :], p[:])
                    nc.sync.dma_start(kvK[h][:, :, :], k_stage[h][:, :, :])
                    nc.gpsimd.collective_compute(
                        "AllGather", mybir.AluOpType.bypass,
                        replica_groups=[list(range(NC))],
                        ins=[kvK[h].opt()], outs=[agK[h].opt()],
                    )

                def q_proj(h):
                    for dci in range(4):
                        dc = 4 * h + dci
                        p = ps1.tile([128, SQ], F32, tag="pp", name=f"qp{dc}")
                        for hc in range(HC):
                            nc.tensor.matmul(
                                p[:], wq_sb[:, hc, dc * 128:(dc + 1) * 128],
                                xT_sb[:, hc, :],
                                start=(hc == 0), stop=(hc == HC - 1))
                        nc.vector.tensor_copy(qT_sb[:, dc, :], p[:])

                def v_proj(hv):
                    for j in range(4):
                        p = ps1.tile([128, HD], F32, tag="pp", name=f"vp{hv}_{j}")
                        nc.tensor.matmul(p[:], onesrow_sb[:, :],
                                         vb_sb[:, hv * HD:(hv + 1) * HD],
                                         start=True, stop=False)
                        for hc in range(HC):
                            nc.tensor.matmul(
                                p[:], xT_sb[:, hc, j * 128:(j + 1) * 128],
                                wv_sb[:, hc, hv * HD:(hv + 1) * HD],
                                start=False, stop=(hc == HC - 1))
                        nc.scalar.copy(v_stage[hv][:, j, :], p[:])
                    nc.sync.dma_start(kvV[hv][:, :, :], v_stage[hv][:, :, :])
                    nc.gpsimd.collective_compute(
                        "AllGather", mybir.AluOpType.bypass,
                        replica_groups=[list(range(NC))],
                        ins=[kvV[hv].opt()], outs=[agV[hv].opt()],
                    )

                # interleave so QK h0 unblocks asap and AGs stay ordered
                q_proj(0)
                v_proj(0)
                q_proj(1)
                v_proj(1)
                # wo load late on gpsimd: after the AG triggers, before stage 3
                nc.gpsimd.dma_start(wo_sb, wod[:, :, :])

            # ============ Stage 2: gathered K/V + fp8 attention ==============
            with (
                tc.tile_pool(name="attn", bufs=1) as attn,
                tc.tile_pool(name="ps_s", bufs=2, space="PSUM") as ps_s,
                tc.tile_pool(name="ps_c", bufs=1, space="PSUM") as ps_c,
                tc.tile_pool(name="ps_l", bufs=2, space="PSUM") as ps_l,
            ):
                # copies from agout; kT on sync queue, v on gpsimd queue
                for h in range(2):
                    for c in range(NC):
                        nc.sync.dma_start(
                            kT_full[:, 4 * h:4 * h + 4, c * SQ:(c + 1) * SQ],
                            agK[h][c])
                for hv in range(2):
                    for c in range(NC):
                        nc.gpsimd.dma_start(
                            v_full[:, 4 * c:4 * c + 4, hv * HD:(hv + 1) * HD],
                            agV[hv][c])

                PTs = [attn.tile([128, KC // 2, 2, SQ], F8, tag=f"PT{h}",
                                 name=f"PT{h}") for h in range(2)]

                # QK + exp for both heads first (exp on scalar is the floor)
                for h in range(2):
                    for kc in range(KC):
                        ps = ps_s.tile([128, SQ], F32, tag="st", name=f"st{h}_{kc}")
                        for dcp in range(2):
                            nc.tensor.matmul(
                                ps[:],
                                kT_full[:, 4 * h + 2 * dcp:4 * h + 2 * dcp + 2,
                                        kc * 128:(kc + 1) * 128],
                                qT_sb[:, 4 * h + 2 * dcp:4 * h + 2 * dcp + 2, :],
                                start=(dcp == 0), stop=(dcp == 1), perf_mode=PM)
                        bias_ap = mb_sb[:, kc:kc + 1] if h == 0 else zb_sb
                        nc.scalar.activation(PTs[h][:, kc // 2, kc % 2, :], ps[:],
                                             AF.Exp, bias=bias_ap, scale=SC)

                # PV + lsum per head, ctx accumulated in PSUM
                for h in range(2):
                    ctx_ps = ps_c.tile([128, 4, SQ], F32, tag="ctx", name=f"ctx{h}")
                    lsum = ps_l.tile([128, SQ], F32, tag="ls", name=f"lsum{h}")
                    for kcp in range(KC // 2):
                        nc.tensor.matmul(lsum[:], v_full[:, 2 * kcp:2 * kcp + 2,
                                                         H:H + 128],
                                         PTs[h][:, kcp, :, :],
                                         start=(kcp == 0), stop=(kcp == KC // 2 - 1),
                                         perf_mode=PM, skip_group_check=True)
                        for dv in range(4):
                            nc.tensor.matmul(
                                ctx_ps[:, dv, :],
                                v_full[:, 2 * kcp:2 * kcp + 2,
                                       h * HD + dv * 128:h * HD + (dv + 1) * 128],
                                PTs[h][:, kcp, :, :],
                                start=(kcp == 0), stop=(kcp == KC // 2 - 1),
                                perf_mode=PM, skip_group_check=True)
                    # normalize: lsum is already broadcast across partitions
                    rl_b = rlp.tile([128, SQ], F32, tag="rlb", name=f"rlb{h}")
                    nc.vector.reciprocal(rl_b[:], lsum[:])
                    for dv in range(4):
                        nc.vector.tensor_mul(ctxT[:, 4 * h + dv, :],
                                             ctx_ps[:, dv, :], rl_b[:])

            # ---- Stage 3: bf16 out-proj + residual + LayerNorm ----
            with (
                tc.tile_pool(name="s4", bufs=2) as s4,
                tc.tile_pool(name="ps4", bufs=2, space="PSUM") as ps4,
            ):
                if affine:
                    nc.gpsimd.dma_start(
                        lnw_b, bass.AP(tensor=lnw, offset=0, ap=[[0, 128], [1, H]]))
                    nc.gpsimd.dma_start(
                        lnb_b, bass.AP(tensor=lnb, offset=0, ap=[[0, 128], [1, H]]))
                for qb in range(QB):
                    xq_f = s4.tile([128, H], F32, tag="xqf", name=f"xqf{qb}")
                    nc.sync.dma_start(xq_f, xqd[qb * 128:(qb + 1) * 128, :])
                    res_f = s4.tile([128, H], F32, tag="resf", name=f"resf{qb}")
                    for h2 in range(H // 512):
                        p = ps4.tile([128, 512], F32, tag="pout", name=f"po{qb}_{h2}")
                        for dc in range(HC):
                            nc.tensor.matmul(
                                p[:],
                                ctxT[:, dc, qb * 128:(qb + 1) * 128],
                                wo_sb[:, dc, h2 * 512:(h2 + 1) * 512],
                                start=(dc == 0), stop=(dc == HC - 1))
                        nc.vector.tensor_add(res_f[:, h2 * 512:(h2 + 1) * 512], p[:],
                                             xq_f[:, h2 * 512:(h2 + 1) * 512])
                    # LayerNorm via bn_stats
                    LS = s4.tile([128, 16], F32, tag="lns", name=f"lns{qb}")
                    for h2 in range(H // 512):
                        nc.vector.bn_stats(
                            LS[:, h2 * 6:(h2 + 1) * 6]
                            .rearrange("p (a b) -> p a b", a=1),
                            res_f[:, h2 * 512:(h2 + 1) * 512])
                    nc.vector.bn_aggr(LS[:, 12:14], LS[:, 0:12]
                                      .rearrange("p (a b) -> p a b", a=2))
                    nc.scalar.activation(LS[:, 14:15], LS[:, 13:14], AF.Sqrt,
                                         bias=eps_sb)
                    nc.vector.reciprocal(LS[:, 15:16], LS[:, 14:15])
                    fin = s4.tile([128, H], F32, tag="fin", name=f"fin{qb}")
                    for h2 in range(H // 512):
                        sl = slice(h2 * 512, (h2 + 1) * 512)
                        nc.vector.tensor_scalar(fin[:, sl], res_f[:, sl],
                                                LS[:, 12:13], LS[:, 15:16],
                                                ALU.subtract, ALU.mult)
                        if affine:
                            nc.vector.tensor_mul(fin[:, sl], fin[:, sl], lnw_b[:, sl])
                            nc.vector.tensor_add(fin[:, sl], fin[:, sl], lnb_b[:, sl])
                        nc.sync.dma_start(out[qb * 128:(qb + 1) * 128, sl],
                                          fin[:, sl])

    nc.compile()
    return nc


_CACHED_NC = {}


def _get_nc(affine: bool):
    if affine not in _CACHED_NC:
        _CACHED_NC[affine] = build_program(affine)
    return _CACHED_NC[affine]


def _pack_T(a):
    """[R, C] f32 -> [128, R//128, C] bf16 with [p, rc, c] = a[rc*128+p, c]."""
    R, C = a.shape
    t = np.asarray(a, np.float32).reshape(R // 128, 128, C)
    return np.ascontiguousarray(t.transpose(1, 0, 2)).astype(ml_dtypes.bfloat16)


def _prep_inputs(inputs, static_data, base_mask, Wq, Wk, Wv, Wo, Ws, bs, ln_w, ln_b):
    f32 = np.float32
    f8 = ml_dtypes.float8_e4m3fn
    bf = ml_dtypes.bfloat16
    x = np.asarray(inputs, f32)
    semb = np.asarray(Ws, f32) @ np.asarray(static_data, f32) + np.asarray(bs, f32)
    vb = np.asarray(Wv, f32) @ semb
    affine = not (np.all(np.asarray(ln_w, f32) == 1.0)
                  and np.all(np.asarray(ln_b, f32) == 0.0))
    common = {
        "wqd": _pack_T(np.asarray(Wq, f32).T),   # [din, dout] packed
        "wkd": _pack_T(np.asarray(Wk, f32).T),
        "wvd": _pack_T(np.asarray(Wv, f32).T),
        "wod": _pack_T(np.asarray(Wo, f32).T),
        "vbd": np.ascontiguousarray(vb.reshape(1, H)).astype(bf),
        "mbias": np.ascontiguousarray(
            np.where(np.asarray(base_mask, bool), -MSHIFT, -1e30)
            .astype(f32).reshape(KC, 128).T),
        "onesrowd": np.ones((1, 128), bf),
    }
    if affine:
        common["lnw"] = np.ascontiguousarray(ln_w, f32)
        common["lnb"] = np.ascontiguousarray(ln_b, f32)
    in_maps = []
    for c in range(NC):
        rows = slice(c * SQ, (c + 1) * SQ)
        m = dict(common)
        m["xTd"] = _pack_T(x[rows].T)
        m["xqd"] = np.ascontiguousarray(x[rows])
        in_maps.append(m)
    return in_maps, affine


def kernel_run(trace=False, **inputs):
    in_maps, affine = _prep_inputs(**inputs)
    nc = _get_nc(affine)
    res = run_bass_kernel_spmd(nc, in_maps, core_ids=list(range(NC)), trace=trace)
    outp = np.concatenate([res.results[c]["out"] for c in range(NC)], axis=0)
    return outp, res


def kernel(**inputs):
    outp, _ = kernel_run(trace=False, **inputs)
    return outp


# revision 16
# speedup vs baseline: 1.6338x; 1.0750x over previous
"""Trainium2 Bass kernel for nn_AttentionBlock (S=4096, H=1024, NH=2, DS=64).

Strategy (v5): sequence parallelism; bf16 projections with fp8 (e4m3)
DoubleRow attention, head-0 key packing, and two merged fp8 AllGathers.

Each core owns 512 rows (queries AND keys). Head 0's mask depends only on
the KEY, so masked keys are dropped exactly: the host packs each core's
unmasked keys (<=263 of 512) into a 384-slot block of x^T; padding slots
carry a -1e30 exp bias so they contribute nothing. Head 1 uses all keys.

Host ships x^T / packed x^T / weights in bf16 in the exact SBUF layouts
plus vb = Wv@semb (the K-side semb constant cancels in softmax; the V-side
enters as a ones-row matmul into the V psum group). Projection PSUM
results are quantized to fp8 on copy-out. K^T(packed h0 + full h1) and
V(packed h0-half + full h1-half) ship as ONE AllGather each (~3.4MB out,
fp8) triggered as soon as their projections finish; no warm-up collective
(the first AG absorbs the cross-core rendezvous). gpsimd issues all its
DMAs before any AG trigger because collective_compute blocks that queue.

QK, PV and the lsum run as fp8 DoubleRow matmuls; exp(logits - 3.5) on the
scalar engine stays inside e4m3 range (exact softmax invariance). A ones
block appended to the V tiles makes the lsum a regular DoubleRow matmul
whose PSUM result is already broadcast across partitions for the
normalize. ctx accumulates in PSUM, is normalized to bf16, and the
out-projection + residual + LayerNorm run in bf16/f32 as in the reference.
"""

import math
import sys

sys.path.insert(0, "/opt/trn_rl_repo")

import numpy as np
import ml_dtypes

import concourse.bass as bass
import concourse.mybir as mybir
import concourse.tile as tile
from concourse import bacc
from concourse.bass_utils import run_bass_kernel_spmd

S, H, NH, DS = 4096, 1024, 2, 64
HD = H // NH            # 512
NC = 8                  # cores
SQ = S // NC            # 512 queries (and keys) per core
EPS = 1e-5
F32 = mybir.dt.float32
BF16 = mybir.dt.bfloat16
F8 = mybir.dt.float8e4
AF = mybir.ActivationFunctionType
ALU = mybir.AluOpType
PM = mybir.MatmulPerfMode.DoubleRow

KC = S // 128           # 32 global key chunks of 128 (head 1)
HC = H // 128           # 8 hidden chunks of 128
QB = SQ // 128          # 4 query chunks of 128
SC = 1.0 / math.sqrt(HD)               # exp input scale
MSHIFT = 3.5            # uniform logit shift: keeps exp() in fp8 range (exact)


def build_program(affine: bool, nk0: int):
    nj0 = nk0 // 128            # packed head-0 key chunks per core
    k0c = NC * nj0              # global packed head-0 key chunks
    k0g = NC * nk0              # global packed head-0 keys
    agw_k = 4 * nk0 + 4 * 512   # AG payload cols (K): h0 packed + h1 full
    agw_v = nj0 * 512 + 4 * 512  # AG payload cols (V)
    off_k1 = 4 * nk0
    off_v1 = nj0 * 512

    nc = bacc.Bacc("TRN2", target_bir_lowering=False, debug=False, num_devices=NC)

    # ---- DRAM I/O (host pre-arranged layouts) ----
    xTd = nc.dram_tensor("xTd", [128, HC, SQ], BF16, kind="ExternalInput")
    xTpd = nc.dram_tensor("xTpd", [128, HC, nk0], BF16, kind="ExternalInput")
    wqd = nc.dram_tensor("wqd", [128, HC, H], BF16, kind="ExternalInput")
    wkd = nc.dram_tensor("wkd", [128, HC, H], BF16, kind="ExternalInput")
    wvd = nc.dram_tensor("wvd", [128, HC, H], BF16, kind="ExternalInput")
    wod = nc.dram_tensor("wod", [128, HC, H], BF16, kind="ExternalInput")
    vbd = nc.dram_tensor("vbd", [1, H], BF16, kind="ExternalInput")
    xqd = nc.dram_tensor("xqd", [SQ, H], F32, kind="ExternalInput")
    mb0d = nc.dram_tensor("mb0d", [128, k0c], F32, kind="ExternalInput")
    onesrowd = nc.dram_tensor("onesrowd", [1, 128], BF16, kind="ExternalInput")
    if affine:
        lnw = nc.dram_tensor("lnw", [H], F32, kind="ExternalInput")
        lnb = nc.dram_tensor("lnb", [H], F32, kind="ExternalInput")
    out = nc.dram_tensor("out", [SQ, H], F32, kind="ExternalOutput")

    with tile.TileContext(nc) as tc:
        with (
            tc.tile_pool(name="consts", bufs=1) as consts,
            tc.tile_pool(name="persist", bufs=1) as persist,
            tc.tile_pool(name="rlp", bufs=1) as rlp,
            tc.tile_pool(name="dram", bufs=1, space="DRAM") as dram,
        ):
            # ---- constants (gpsimd DMAs all BEFORE any AG trigger) ----
            Af = consts.tile([128, k0c + 2], F32)   # 0:k0c mb0 | -M | eps
            mb0 = Af[:, 0:k0c]
            nc.gpsimd.dma_start(mb0, mb0d[:, :])
            zb_sb = Af[:, k0c:k0c + 1]
            nc.vector.memset(zb_sb, -MSHIFT)
            eps_sb = Af[:, k0c + 1:k0c + 2]
            nc.vector.memset(eps_sb, EPS)
            onesrow_sb = consts.tile([1, 128], BF16)
            nc.gpsimd.dma_start(onesrow_sb, onesrowd[:, :])
            vb_sb = consts.tile([1, H], BF16)
            nc.gpsimd.dma_start(vb_sb, vbd[:, :])
            if affine:
                lnw_b = consts.tile([128, H], F32)
                lnb_b = consts.tile([128, H], F32)
                nc.sync.dma_start(
                    lnw_b, bass.AP(tensor=lnw, offset=0, ap=[[0, 128], [1, H]]))
                nc.sync.dma_start(
                    lnb_b, bass.AP(tensor=lnb, offset=0, ap=[[0, 128], [1, H]]))

            # ---- persistent tiles ----
            xT_sb = persist.tile([128, HC, SQ], BF16)
            xTp_sb = persist.tile([128, HC, nk0], BF16)
            wq_sb = persist.tile([128, HC, H], BF16)
            wo_sb = persist.tile([128, HC, H], BF16)
            qT_sb = persist.tile([128, HC, SQ], F8)      # Q^T fp8 [d, q]
            kT0 = persist.tile([128, 4, k0g], F8)        # K^T h0 packed
            kT1 = persist.tile([128, 4, S], F8)          # K^T h1 full
            v0 = persist.tile([128, k0c, HD + 128], F8)  # V h0 packed + ones
            v1 = persist.tile([128, KC, HD + 128], F8)   # V h1 full + ones
            ctxT = persist.tile([128, HC, SQ], BF16)     # ctx^T/l bf16 [d, q]

            nc.vector.memset(v0[:, :, HD:HD + 128], 1.0)
            nc.vector.memset(v1[:, :, HD:HD + 128], 1.0)

            # DRAM scratch for collectives
            kvK = dram.tile([128, agw_k], F8, name="kvK")
            agKo = dram.tile([NC, 128, agw_k], F8, addr_space="Shared",
                             name="agKo")
            kvV = dram.tile([128, agw_v], F8, name="kvV")
            agVo = dram.tile([NC, 128, agw_v], F8, addr_space="Shared",
                             name="agVo")

            # ================= Stage 1: own-block projections =================
            # K projects plain x: the Wk@semb key-constant adds a per-query
            # offset to the logits which cancels in softmax. V gets Wv@semb
            # via a ones-row matmul with the host-computed vb row.
            with (
                tc.tile_pool(name="s1", bufs=1) as s1,
                tc.tile_pool(name="ps1", bufs=3, space="PSUM") as ps1,
            ):
                wk_sb = s1.tile([128, HC, H], BF16, tag="wk", name="wk_sb")
                wv_sb = s1.tile([128, HC, H], BF16, tag="wv", name="wv_sb")
                k_stage = s1.tile([128, agw_k], F8, tag="ks", name="k_stage")
                v_stage = s1.tile([128, agw_v], F8, tag="vs", name="v_stage")

                # loads; wk/xT first (K^T gates the first AllGather)
                nc.sync.dma_start(xTp_sb, xTpd[:, :, :])
                nc.sync.dma_start(xT_sb, xTd[:, :, :])
                nc.scalar.dma_start(wk_sb[:, :, 0:HD], wkd[:, :, 0:HD])
                nc.scalar.dma_start(wk_sb[:, :, HD:H], wkd[:, :, HD:H])
                nc.gpsimd.dma_start(wv_sb[:, :, 0:HD], wvd[:, :, 0:HD])
                nc.gpsimd.dma_start(wv_sb[:, :, HD:H], wvd[:, :, HD:H])
                nc.sync.dma_start(wq_sb[:, :, 0:HD], wqd[:, :, 0:HD])
                nc.sync.dma_start(wq_sb[:, :, HD:H], wqd[:, :, HD:H])
                nc.gpsimd.dma_start(wo_sb, wod[:, :, :])

                # K^T: h0 from packed x, h1 from full x
                for dc in range(HC):
                    w = nk0 if dc < 4 else SQ
                    src = xTp_sb if dc < 4 else xT_sb
                    p = ps1.tile([128, SQ], F32, tag="pp", name=f"kp{dc}")
                    for hc in range(HC):
                        nc.tensor.matmul(
                            p[:, 0:w], wk_sb[:, hc, dc * 128:(dc + 1) * 128],
                            src[:, hc, :],
                            start=(hc == 0), stop=(hc == HC - 1))
                    if dc < 4:
                        dst = k_stage[:, dc * nk0:(dc + 1) * nk0]
                    else:
                        dst = k_stage[:, off_k1 + (dc - 4) * 512:
                                      off_k1 + (dc - 3) * 512]
                    nc.scalar.copy(dst, p[:, 0:w])
                nc.sync.dma_start(kvK[:, :], k_stage[:, :])
                nc.gpsimd.collective_compute(
                    "AllGather", mybir.AluOpType.bypass,
                    replica_groups=[list(range(NC))],
                    ins=[kvK.opt()], outs=[agKo.opt()],
                )

                # V: h0 d-half from packed x, h1 d-half from full x
                for hv in range(2):
                    njs = nj0 if hv == 0 else 4
                    src = xTp_sb if hv == 0 else xT_sb
                    off = 0 if hv == 0 else off_v1
                    for j in range(njs):
                        p = ps1.tile([128, HD], F32, tag="pp", name=f"vp{hv}_{j}")
                        nc.tensor.matmul(p[:], onesrow_sb[:, :],
                                         vb_sb[:, hv * HD:(hv + 1) * HD],
                                         start=True, stop=False)
                        for hc in range(HC):
                            nc.tensor.matmul(
                                p[:], src[:, hc, j * 128:(j + 1) * 128],
                                wv_sb[:, hc, hv * HD:(hv + 1) * HD],
                                start=False, stop=(hc == HC - 1))
                        nc.scalar.copy(
                            v_stage[:, off + j * 512:off + (j + 1) * 512], p[:])
                nc.sync.dma_start(kvV[:, :], v_stage[:, :])
                nc.gpsimd.collective_compute(
                    "AllGather", mybir.AluOpType.bypass,
                    replica_groups=[list(range(NC))],
                    ins=[kvV.opt()], outs=[agVo.opt()],
                )

                # Q^T (overlaps the AllGathers)
                for dc in range(HC):
                    p = ps1.tile([128, SQ], F32, tag="pp", name=f"qp{dc}")
                    for hc in range(HC):
                        nc.tensor.matmul(
                            p[:], wq_sb[:, hc, dc * 128:(dc + 1) * 128],
                            xT_sb[:, hc, :],
                            start=(hc == 0), stop=(hc == HC - 1))
                    nc.vector.tensor_copy(qT_sb[:, dc, :], p[:])

            # ============ Stage 2: gathered K/V + fp8 attention ==============
            with (
                tc.tile_pool(name="attn", bufs=1) as attn,
                tc.tile_pool(name="ps_s", bufs=2, space="PSUM") as ps_s,
                tc.tile_pool(name="ps_c", bufs=1, space="PSUM") as ps_c,
                tc.tile_pool(name="ps_l", bufs=2, space="PSUM") as ps_l,
            ):
                # copies from agout; kT on sync queue, v on gpsimd queue
                for c in range(NC):
                    nc.sync.dma_start(
                        kT0[:, :, c * nk0:(c + 1) * nk0],
                        agKo[c][:, 0:off_k1].rearrange("p (a b) -> p a b", a=4))
                for c in range(NC):
                    nc.sync.dma_start(
                        kT1[:, :, c * SQ:(c + 1) * SQ],
                        agKo[c][:, off_k1:agw_k].rearrange("p (a b) -> p a b", a=4))
                for c in range(NC):
                    nc.gpsimd.dma_start(
                        v0[:, c * nj0:(c + 1) * nj0, 0:HD],
                        agVo[c][:, 0:off_v1].rearrange("p (a b) -> p a b", a=nj0))
                for c in range(NC):
                    nc.gpsimd.dma_start(
                        v1[:, 4 * c:4 * c + 4, 0:HD],
                        agVo[c][:, off_v1:agw_v].rearrange("p (a b) -> p a b", a=4))

                kcs = [k0c, KC]
                kts = [kT0, kT1]
                vts = [v0, v1]
                PTs = [attn.tile([128, kcs[h] // 2, 2, SQ], F8, tag=f"PT{h}",
                                 name=f"PT{h}") for h in range(2)]

                # QK + exp for both heads first (exp on scalar is the floor)
                for h in range(2):
                    for kc in range(kcs[h]):
                        ps = ps_s.tile([128, SQ], F32, tag="st", name=f"st{h}_{kc}")
                        for dcp in range(2):
                            nc.tensor.matmul(
                                ps[:],
                                kts[h][:, 2 * dcp:2 * dcp + 2,
                                       kc * 128:(kc + 1) * 128],
                                qT_sb[:, 4 * h + 2 * dcp:4 * h + 2 * dcp + 2, :],
                                start=(dcp == 0), stop=(dcp == 1), perf_mode=PM)
                        bias_ap = mb0[:, kc:kc + 1] if h == 0 else zb_sb
                        nc.scalar.activation(PTs[h][:, kc // 2, kc % 2, :], ps[:],
                                             AF.Exp, bias=bias_ap, scale=SC)

                # PV + lsum per head, ctx accumulated in PSUM
                for h in range(2):
                    kcp_n = kcs[h] // 2
                    vt = vts[h]
                    ctx_ps = ps_c.tile([128, 4, SQ], F32, tag="ctx", name=f"ctx{h}")
                    lsum = ps_l.tile([128, SQ], F32, tag="ls", name=f"lsum{h}")
                    for kcp in range(kcp_n):
                        nc.tensor.matmul(lsum[:],
                                         vt[:, 2 * kcp:2 * kcp + 2, HD:HD + 128],
                                         PTs[h][:, kcp, :, :],
                                         start=(kcp == 0), stop=(kcp == kcp_n - 1),
                                         perf_mode=PM, skip_group_check=True)
                        for dv in range(4):
                            nc.tensor.matmul(
                                ctx_ps[:, dv, :],
                                vt[:, 2 * kcp:2 * kcp + 2,
                                   dv * 128:(dv + 1) * 128],
                                PTs[h][:, kcp, :, :],
                                start=(kcp == 0), stop=(kcp == kcp_n - 1),
                                perf_mode=PM, skip_group_check=True)
                    # normalize: lsum is already broadcast across partitions
                    rl_b = rlp.tile([128, SQ], F32, tag="rlb", name=f"rlb{h}")
                    nc.vector.reciprocal(rl_b[:], lsum[:])
                    for dv in range(4):
                        nc.vector.tensor_mul(ctxT[:, 4 * h + dv, :],
                                             ctx_ps[:, dv, :], rl_b[:])

            # ---- Stage 3: bf16 out-proj + residual + LayerNorm ----
            with (
                tc.tile_pool(name="s4", bufs=2) as s4,
                tc.tile_pool(name="ps4", bufs=2, space="PSUM") as ps4,
            ):
                for qb in range(QB):
                    xq_f = s4.tile([128, H], F32, tag="xqf", name=f"xqf{qb}")
                    nc.sync.dma_start(xq_f, xqd[qb * 128:(qb + 1) * 128, :])
                    res_f = s4.tile([128, H], F32, tag="resf", name=f"resf{qb}")
                    for h2 in range(H // 512):
                        p = ps4.tile([128, 512], F32, tag="pout", name=f"po{qb}_{h2}")
                        for dc in range(HC):
                            nc.tensor.matmul(
                                p[:],
                                ctxT[:, dc, qb * 128:(qb + 1) * 128],
                                wo_sb[:, dc, h2 * 512:(h2 + 1) * 512],
                                start=(dc == 0), stop=(dc == HC - 1))
                        nc.vector.tensor_add(res_f[:, h2 * 512:(h2 + 1) * 512], p[:],
                                             xq_f[:, h2 * 512:(h2 + 1) * 512])
                    # LayerNorm via bn_stats
                    LS = s4.tile([128, 16], F32, tag="lns", name=f"lns{qb}")
                    for h2 in range(H // 512):
                        nc.vector.bn_stats(
                            LS[:, h2 * 6:(h2 + 1) * 6]
                            .rearrange("p (a b) -> p a b", a=1),
                            res_f[:, h2 * 512:(h2 + 1) * 512])
                    nc.vector.bn_aggr(LS[:, 12:14], LS[:, 0:12]
                                      .rearrange("p (a b) -> p a b", a=2))
                    nc.scalar.activation(LS[:, 14:15], LS[:, 13:14], AF.Sqrt,
                                         bias=eps_sb)
                    nc.vector.reciprocal(LS[:, 15:16], LS[:, 14:15])
                    fin = s4.tile([128, H], F32, tag="fin", name=f"fin{qb}")
                    for h2 in range(H // 512):
                        sl = slice(h2 * 512, (h2 + 1) * 512)
                        nc.vector.tensor_scalar(fin[:, sl], res_f[:, sl],
                                                LS[:, 12:13], LS[:, 15:16],
                                                ALU.subtract, ALU.mult)
                        if affine:
                            nc.vector.tensor_mul(fin[:, sl], fin[:, sl], lnw_b[:, sl])
                            nc.vector.tensor_add(fin[:, sl], fin[:, sl], lnb_b[:, sl])
                        nc.sync.dma_start(out[qb * 128:(qb + 1) * 128, sl],
                                          fin[:, sl])

    nc.compile()
    return nc


_CACHED_NC = {}


def _get_nc(affine: bool, nk0: int):
    key = (affine, nk0)
    if key not in _CACHED_NC:
        _CACHED_NC[key] = build_program(affine, nk0)
    return _CACHED_NC[key]


def _pack_T(a):
    """[R, C] f32 -> [128, R//128, C] bf16 with [p, rc, c] = a[rc*128+p, c]."""
    R, C = a.shape
    t = np.asarray(a, np.float32).reshape(R // 128, 128, C)
    return np.ascontiguousarray(t.transpose(1, 0, 2)).astype(ml_dtypes.bfloat16)


def _prep_inputs(inputs, static_data, base_mask, Wq, Wk, Wv, Wo, Ws, bs, ln_w, ln_b):
    f32 = np.float32
    bf = ml_dtypes.bfloat16
    x = np.asarray(inputs, f32)
    mask = np.asarray(base_mask, bool)
    semb = np.asarray(Ws, f32) @ np.asarray(static_data, f32) + np.asarray(bs, f32)
    vb = np.asarray(Wv, f32) @ semb
    affine = not (np.all(np.asarray(ln_w, f32) == 1.0)
                  and np.all(np.asarray(ln_b, f32) == 0.0))

    idxs = [np.where(mask[c * SQ:(c + 1) * SQ])[0] for c in range(NC)]
    maxn = max(len(ix) for ix in idxs)
    nk0 = 384 if maxn <= 384 else 512
    k0c = NC * (nk0 // 128)

    # packed head-0 exp bias: -MSHIFT for real keys, -1e30 for padding
    mb0 = np.full((NC, nk0), -1e30, f32)
    for c in range(NC):
        mb0[c, :len(idxs[c])] = -MSHIFT
    mb0 = mb0.reshape(k0c, 128).T

    common = {
        "wqd": _pack_T(np.asarray(Wq, f32).T),   # [din, dout] packed
        "wkd": _pack_T(np.asarray(Wk, f32).T),
        "wvd": _pack_T(np.asarray(Wv, f32).T),
        "wod": _pack_T(np.asarray(Wo, f32).T),
        "vbd": np.ascontiguousarray(vb.reshape(1, H)).astype(bf),
        "mb0d": np.ascontiguousarray(mb0),
        "onesrowd": np.ones((1, 128), bf),
    }
    if affine:
        common["lnw"] = np.ascontiguousarray(ln_w, f32)
        common["lnb"] = np.ascontiguousarray(ln_b, f32)
    in_maps = []
    for c in range(NC):
        rows = slice(c * SQ, (c + 1) * SQ)
        m = dict(common)
        m["xTd"] = _pack_T(x[rows].T)
        xp = np.zeros((nk0, H), f32)
        xp[:len(idxs[c])] = x[rows][idxs[c]]
        m["xTpd"] = _pack_T(xp.T)
        m["xqd"] = np.ascontiguousarray(x[rows])
        in_maps.append(m)
    return in_maps, affine, nk0


def kernel_run(trace=False, **inputs):
    in_maps, affine, nk0 = _prep_inputs(**inputs)
    nc = _get_nc(affine, nk0)
    res = run_bass_kernel_spmd(nc, in_maps, core_ids=list(range(NC)), trace=trace)
    outp = np.concatenate([res.results[c]["out"] for c in range(NC)], axis=0)
    return outp, res


def kernel(**inputs):
    outp, _ = kernel_run(trace=False, **inputs)
    return outp
